# revision 7
# baseline (speedup 1.0000x reference)
"""CapsNet (nn_CapsNetBasic) forward pass as a Bass/Tile kernel on 8 TRN2 cores.

Sharding: 8 cores = 2 batch samples x 4 row-blocks of 32 output rows each.
Every core computes its 32x128-pixel slab end-to-end:
  conv1 (5x5, 1->256, via host-built im2col with fused valid-mask/bias rows)
  primary caps conv (5x5, 256->256, 50-matmul PSUM accumulation chains)
  per-capsule squash (partition-group reductions via 0/1 indicator matmuls)
  seg caps (1x1 conv + sum over 32 input capsules, fused into one matmul pair)
  seg squash, length output, label masking, recon 1x1 convs (16->64->128->1)
No cross-core communication: halos arrive via the host im2col. Routing
softmaxes are constant for these shapes (uniform 1/32 and singleton 1.0), so
routing reduces to the fixed reductions implemented here.
"""

import sys

sys.path.insert(0, "/opt/trn_rl_repo")

import numpy as np
from contextlib import ExitStack

import concourse.bass as bass
import concourse.tile as tile
from concourse import mybir, bacc
from concourse.bass_utils import run_bass_kernel_spmd

F32 = mybir.dt.float32
F32R = mybir.dt.float32r
AF = mybir.ActivationFunctionType

B = 2
H = W = 128
RB = 32          # output rows per core
NBLK = 4         # row blocks per sample
NCORES = 8
RR = RB + 4      # conv1 buffer rows (halo 2 each side)
CW = W + 4       # padded width
AFLAT = RR * CW  # 4752
NPX = RB * W     # 4096 output pixels per core
NT = NPX // 512  # 8 N-tiles (4 output rows each)

INPUT_SHAPES = {
    "A": (27, AFLAT),
    "W1T": (27, 256),
    "WT": (2, 25, 128, 256),
    "YV": (NPX,),
    "PACKR": (128, 370),   # matmul-constant pack (fp32r)
    "PACKF": (128, 9),     # bias/eps pack (fp32)
}

_PROGRAM = None


def _build_program():
    nc = bacc.Bacc("TRN2", target_bir_lowering=False, debug=False, num_devices=NCORES)

    d = {}
    R_INPUTS = {"A", "W1T", "WT", "PACKR"}
    for name, shape in INPUT_SHAPES.items():
        dt = F32R if name in R_INPUTS else F32
        d[name] = nc.dram_tensor(name, list(shape), dt, kind="ExternalInput").ap()
    for name in ("OSEG", "OREC"):
        d[name] = nc.dram_tensor(name, [NPX], F32, kind="ExternalOutput").ap()

    with tile.TileContext(nc) as tc, ExitStack() as ctx:
        pers = ctx.enter_context(tc.tile_pool(name="pers", bufs=1))
        pa = ctx.enter_context(tc.tile_pool(name="act", bufs=3))
        pt16 = ctx.enter_context(tc.tile_pool(name="t16", bufs=2))
        pt1 = ctx.enter_context(tc.tile_pool(name="t1", bufs=2))
        ppc = ctx.enter_context(tc.tile_pool(name="ppc", bufs=2, space="PSUM"))
        pps = ctx.enter_context(tc.tile_pool(name="pps", bufs=4, space="PSUM"))

        # ---- persistent loads ----
        # W1T + packed constants go first on the sync queue (conv1 and the
        # post-pipeline need them early); WT streams on scalar+gpsimd queues
        # immediately; A arrives in column chunks so conv1 starts early.
        W1T = pers.tile([27, 256], F32R, tag="W1T")
        nc.sync.dma_start(W1T[:], d["W1T"][:])
        PACKR = pers.tile([128, 370], F32R, tag="PACKR")
        nc.sync.dma_start(PACKR[:], d["PACKR"][:])
        PACKF = pers.tile([128, 9], F32, tag="PACKF")
        nc.sync.dma_start(PACKF[:], d["PACKF"][:])

        WsT = PACKR[:, 0:16]
        INDSQ = PACKR[:, 16:32]
        IND2 = PACKR[0:16, 32:160]
        WR1T = PACKR[0:16, 160:224]
        WR2T = PACKR[0:64, 224:352]
        WR3T = PACKR[:, 352:353]
        ONES16 = PACKR[0:16, 353:354]
        ONES1x16 = PACKR[0:1, 354:370]
        CB1 = PACKF[:, 0:2]
        ZERO128 = PACKF[:, 2:3]
        BR1 = PACKF[0:64, 3:4]
        BR2 = PACKF[:, 4:5]
        BR3 = PACKF[0:1, 5:6]
        CB2 = PACKF[0:16, 6:7]
        EPS16 = PACKF[0:16, 7:8]
        EPS1 = PACKF[0:1, 8:9]

        WT = pers.tile([128, 2, 25, 256], F32R, tag="WT")
        _dma_engines = [nc.scalar, nc.gpsimd]
        for k in range(2):
            for t in range(25):
                eng = _dma_engines[(k * 25 + t) % len(_dma_engines)]
                eng.dma_start(WT[:, k, t, :], d["WT"][k, t, :, :])

        A = pers.tile([27, AFLAT], F32R, tag="A")
        for off in range(0, AFLAT, 512):
            n = min(512, AFLAT - off)
            nc.sync.dma_start(A[:, off:off + n], d["A"][:, off:off + n])

        C1 = [pers.tile([128, AFLAT], F32R, tag=f"C1_{m}", name=f"C1_{m}")
              for m in range(2)]

        # ---- conv1: 1->256 5x5 via host im2col (25 taps + valid-mask + bias rows) ----
        for m in range(2):
            off = 0
            while off < AFLAT:
                n = min(512, AFLAT - off)
                ps = ppc.tile([128, 512], F32, tag="ppc")
                nc.tensor.matmul(
                    ps[:, :n],
                    W1T[:, m * 128:(m + 1) * 128],
                    A[:, off:off + n],
                    start=True, stop=True,
                )
                nc.scalar.activation(
                    C1[m][:, off:off + n], ps[:, :n], AF.Relu,
                    bias=ZERO128[:], scale=1.0,
                )
                off += n

        C13 = [C1[m][:].rearrange("p (r c) -> p r c", c=CW) for m in range(2)]

        # ---- main loop over 8 N-tiles of 512 px (4 output rows each) ----
        for t in range(NT):
            px = slice(512 * t, 512 * (t + 1))
            prim = []
            for m in range(2):
                # primary caps conv: 50-matmul accumulation chain
                ps = ppc.tile([128, 512], F32, tag="ppc")
                idx = 0
                for k in range(2):
                    for dy in range(5):
                        for dx in range(5):
                            nc.tensor.matmul(
                                ps[:],
                                WT[:, k, dy * 5 + dx, m * 128:(m + 1) * 128],
                                C13[k][:, 4 * t + dy:4 * t + dy + 4, dx:dx + 128],
                                start=(idx == 0), stop=(idx == 49),
                            )
                            idx += 1
                # preact = psum/32 + (bp/32 + cbp)
                P = pa.tile([128, 512], F32, tag="P")
                nc.scalar.activation(P[:], ps[:], AF.Identity,
                                     bias=CB1[:, m:m + 1], scale=1.0 / 32.0)
                # squash over each capsule's 8 atoms (partition groups)
                S = pa.tile([128, 512], F32R, tag="S")
                nc.vector.tensor_mul(out=S[:], in0=P[:], in1=P[:])
                sq = pps.tile([128, 512], F32, tag="pps")
                nc.tensor.matmul(sq[:16, :], INDSQ[:], S[:],
                                 start=True, stop=True)
                tq = pt16.tile([16, 512], F32, tag="tq")
                nc.scalar.activation(tq[:], sq[:16, :], AF.Sqrt, bias=EPS16[:], scale=1.0)
                u = pt16.tile([16, 512], F32, tag="u")
                nc.vector.tensor_mul(out=u[:], in0=sq[:16, :], in1=tq[:])
                nc.vector.tensor_add(out=u[:], in0=u[:], in1=tq[:])
                rf0 = pt16.tile([16, 512], F32, tag="rf0")
                nc.vector.reciprocal_approx_fast(out=rf0[:], in_=u[:])
                rf = pt16.tile([16, 512], F32R, tag="rf")
                nc.vector.tensor_mul(out=rf[:], in0=sq[:16, :], in1=rf0[:])
                bc = pps.tile([128, 512], F32, tag="pps")
                nc.tensor.matmul(bc[:], IND2[:], rf[:],
                                 start=True, stop=True)
                pm = pa.tile([128, 512], F32R, tag="prim")
                nc.vector.tensor_mul(out=pm[:], in0=P[:], in1=bc[:])
                prim.append(pm)

            # seg votes + sum over 32 input capsules, both chunks into one psum
            spp = pps.tile([128, 512], F32, tag="pps")
            nc.tensor.matmul(spp[:16, :], WsT[:], prim[0][:],
                             start=True, stop=False)
            nc.tensor.matmul(spp[:16, :], WsT[:], prim[1][:],
                             start=False, stop=True)
            sp = pt16.tile([16, 512], F32, tag="sp")
            nc.scalar.activation(sp[:], spp[:16, :], AF.Identity, bias=CB2[:], scale=1.0)

            # seg squash scalar factor from sq2 = sum_a sp^2
            sp2 = pt16.tile([16, 512], F32R, tag="sp2")
            nc.vector.tensor_mul(out=sp2[:], in0=sp[:], in1=sp[:])
            sq2p = pps.tile([128, 512], F32, tag="pps")
            nc.tensor.matmul(sq2p[:1, :], ONES16[:], sp2[:],
                             start=True, stop=True)
            t2 = pt1.tile([1, 512], F32, tag="t2")
            nc.scalar.activation(t2[:], sq2p[:1, :], AF.Sqrt, bias=EPS1[:], scale=1.0)
            u2 = pt1.tile([1, 512], F32, tag="u2")
            nc.vector.tensor_mul(out=u2[:], in0=sq2p[:1, :], in1=t2[:])
            nc.vector.tensor_add(out=u2[:], in0=u2[:], in1=t2[:])
            f2 = pt1.tile([1, 512], F32, tag="f2")
            nc.vector.reciprocal_approx_fast(out=f2[:], in_=u2[:])
            nc.vector.tensor_mul(out=f2[:], in0=sq2p[:1, :], in1=f2[:])

            # out_seg = sqrt(f2^2 * sq2 + eps)
            q = pt1.tile([1, 512], F32, tag="q")
            nc.vector.tensor_mul(out=q[:], in0=f2[:], in1=f2[:])
            nc.vector.tensor_mul(out=q[:], in0=q[:], in1=sq2p[:1, :])
            oseg = pt1.tile([1, 512], F32, tag="oseg")
            nc.scalar.activation(oseg[:], q[:], AF.Sqrt, bias=EPS1[:], scale=1.0)
            nc.sync.dma_start(d["OSEG"][px].rearrange("(p n) -> p n", p=1), oseg[:])

            # masked = sp * (f2 * y), broadcast over the 16 atoms
            yt = pt1.tile([1, 512], F32, tag="yt")
            nc.sync.dma_start(yt[:], d["YV"][px].rearrange("(p n) -> p n", p=1))
            m1 = pt1.tile([1, 512], F32R, tag="m1")
            nc.vector.tensor_mul(out=m1[:], in0=f2[:], in1=yt[:])
            bmp = pps.tile([128, 512], F32, tag="pps")
            nc.tensor.matmul(bmp[:16, :], ONES1x16[:], m1[:],
                             start=True, stop=True)
            masked = pt16.tile([16, 512], F32R, tag="masked")
            nc.vector.tensor_mul(out=masked[:], in0=sp[:], in1=bmp[:16, :])

            # recon: 16 -> 64 -> 128 -> 1 (1x1 convs)
            r1p = pps.tile([128, 512], F32, tag="pps")
            nc.tensor.matmul(r1p[:64, :], WR1T[:], masked[:],
                             start=True, stop=True)
            r1 = pa.tile([64, 512], F32R, tag="r1")
            nc.scalar.activation(r1[:], r1p[:64, :], AF.Relu, bias=BR1[:], scale=1.0)
            r2p = pps.tile([128, 512], F32, tag="pps")
            nc.tensor.matmul(r2p[:], WR2T[:], r1[:],
                             start=True, stop=True)
            r2 = pa.tile([128, 512], F32R, tag="r2")
            nc.scalar.activation(r2[:], r2p[:], AF.Relu, bias=BR2[:], scale=1.0)
            r3p = pps.tile([128, 512], F32, tag="pps")
            nc.tensor.matmul(r3p[:1, :], WR3T[:], r2[:],
                             start=True, stop=True)
            orec = pt1.tile([1, 512], F32, tag="orec")
            nc.scalar.activation(orec[:], r3p[:1, :], AF.Sigmoid, bias=BR3[:], scale=1.0)
            nc.sync.dma_start(d["OREC"][px].rearrange("(p n) -> p n", p=1), orec[:])

    nc.compile()
    return nc


def _get_program():
    global _PROGRAM
    if _PROGRAM is None:
        _PROGRAM = _build_program()
    return _PROGRAM


def _host_prep(inputs):
    """Build per-core input maps from the full problem inputs."""
    x = np.asarray(inputs["x"], np.float32)
    y = np.asarray(inputs["y"], np.float32)
    W1 = np.asarray(inputs["W1"], np.float32)
    b1 = np.asarray(inputs["b1"], np.float32)
    Wp = np.asarray(inputs["Wp"], np.float32)
    bp = np.asarray(inputs["bp"], np.float32)
    cbp = np.asarray(inputs["cbp"], np.float32)
    Ws = np.asarray(inputs["Ws"], np.float32)
    bs = np.asarray(inputs["bs"], np.float32)
    cbs = np.asarray(inputs["cbs"], np.float32)
    Wr1 = np.asarray(inputs["Wr1"], np.float32)
    br1 = np.asarray(inputs["br1"], np.float32)
    Wr2 = np.asarray(inputs["Wr2"], np.float32)
    br2 = np.asarray(inputs["br2"], np.float32)
    Wr3 = np.asarray(inputs["Wr3"], np.float32)
    br3 = np.asarray(inputs["br3"], np.float32)

    W1r = W1.reshape(256, 25).T                      # [25 tap, 256 oc]
    W1T = np.concatenate([W1r, np.ones((1, 256), np.float32),
                          b1[None, :]], axis=0)      # [27, 256]
    WT = np.ascontiguousarray(
        Wp.reshape(256, 2, 128, 25).transpose(1, 3, 2, 0))  # [2k, 25tap, 128p, 256oc]

    oc = np.arange(128)
    WsT = np.ascontiguousarray(Ws.reshape(16, 8).T[oc % 8])       # [128, 16]
    IND2 = (np.arange(128)[None, :] // 8 == np.arange(16)[:, None]).astype(np.float32)
    INDSQ = np.ascontiguousarray(IND2.T)
    cb1 = np.empty((128, 2), np.float32)
    for m in range(2):
        g = m * 128 + np.arange(128)
        cb1[:, m] = bp[g] / 32.0 + cbp[g // 8, g % 8, 0, 0]
    cb2 = (32.0 * bs + cbs[0, :, 0, 0]).astype(np.float32)[:, None]

    packr = np.zeros((128, 370), np.float32)
    packr[:, 0:16] = WsT
    packr[:, 16:32] = INDSQ
    packr[0:16, 32:160] = IND2
    packr[0:16, 160:224] = Wr1.reshape(64, 16).T
    packr[0:64, 224:352] = Wr2.reshape(128, 64).T
    packr[:, 352:353] = Wr3.reshape(1, 128).T
    packr[0:16, 353:354] = 1.0
    packr[0:1, 354:370] = 1.0
    packf = np.zeros((128, 9), np.float32)
    packf[:, 0:2] = cb1
    packf[0:64, 3] = br1
    packf[:, 4] = br2
    packf[0, 5] = br3[0]
    packf[0:16, 6] = cb2[:, 0]
    packf[0:16, 7] = 1e-9
    packf[0, 8] = 1e-9
    shared = {
        "W1T": np.ascontiguousarray(W1T),
        "WT": WT,
        "PACKR": packr,
        "PACKF": packf,
    }

    in_maps = []
    for c in range(NCORES):
        b, j = divmod(c, NBLK)
        r0 = RB * j
        xpad = np.zeros((H + 8, W + 8), np.float32)
        xpad[4:4 + H, 4:4 + W] = x[b, 0]
        A = np.empty((27, RR, CW), np.float32)
        for dy in range(5):
            for dx in range(5):
                A[dy * 5 + dx] = xpad[r0 + dy:r0 + dy + RR, dx:dx + CW]
        # valid-mask row: -1e30 where the conv1 output position is padding
        rr = np.arange(RR)[:, None]
        cc = np.arange(CW)[None, :]
        valid = (r0 - 2 + rr >= 0) & (r0 - 2 + rr < H) & (cc >= 2) & (cc < 2 + W)
        A[25] = np.where(valid, 0.0, -1e30).astype(np.float32)
        A[26] = 1.0
        m = dict(shared)
        m["A"] = np.ascontiguousarray(A.reshape(27, AFLAT))
        m["YV"] = np.ascontiguousarray(y[b, 0, r0:r0 + RB, :].reshape(NPX))
        in_maps.append(m)
    return in_maps


def _gather(results):
    out_seg = np.empty((B, 1, H, W), np.float32)
    out_rec = np.empty((B, 1, H, W), np.float32)
    for c in range(NCORES):
        b, j = divmod(c, NBLK)
        r0 = RB * j
        out_seg[b, 0, r0:r0 + RB, :] = results[c]["OSEG"].reshape(RB, W)
        out_rec[b, 0, r0:r0 + RB, :] = results[c]["OREC"].reshape(RB, W)
    return out_seg, out_rec


def kernel(**inputs):
    nc = _get_program()
    in_maps = _host_prep(inputs)
    res = run_bass_kernel_spmd(nc, in_maps, list(range(NCORES)))
    return _gather(res.results)


# revision 9
# speedup vs baseline: 1.0350x; 1.0350x over previous
"""CapsNet (nn_CapsNetBasic) forward pass as a Bass/Tile kernel on 8 TRN2 cores.

Sharding: 8 cores = 2 batch samples x 4 row-blocks of 32 output rows each.
Every core computes its 32x128-pixel slab end-to-end:
  conv1 (5x5, 1->256, via host-built im2col with fused valid-mask/bias rows)
  primary caps conv (5x5, 256->256, 50-matmul PSUM accumulation chains)
  per-capsule squash (partition-group reductions via 0/1 indicator matmuls)
  seg caps (1x1 conv + sum over 32 input capsules, fused into one matmul pair)
  seg squash, length output, label masking, recon 1x1 convs (16->64->128->1)
No cross-core communication: halos arrive via the host im2col. Routing
softmaxes are constant for these shapes (uniform 1/32 and singleton 1.0), so
routing reduces to the fixed reductions implemented here.
"""

import sys

sys.path.insert(0, "/opt/trn_rl_repo")

import numpy as np
from contextlib import ExitStack

import concourse.bass as bass
import concourse.tile as tile
from concourse import mybir, bacc
from concourse.bass_utils import run_bass_kernel_spmd

F32 = mybir.dt.float32
F32R = mybir.dt.float32r
AF = mybir.ActivationFunctionType

B = 2
H = W = 128
RB = 32          # output rows per core
NBLK = 4         # row blocks per sample
NCORES = 8
RR = RB + 4      # conv1 buffer rows (halo 2 each side)
CW = W + 4       # padded width
AFLAT = RR * CW  # 4752
NPX = RB * W     # 4096 output pixels per core
NT = NPX // 512  # 8 N-tiles (4 output rows each)

INPUT_SHAPES = {
    "A4": (128, AFLAT // 4),
    "W1T4": (128, 256),
    "WT": (2, 25, 128, 256),
    "YV": (NPX,),
    "PACKR": (128, 370),   # matmul-constant pack (fp32r)
    "PACKF": (128, 9),     # bias/eps pack (fp32)
}

_PROGRAM = None


def _build_program():
    nc = bacc.Bacc("TRN2", target_bir_lowering=False, debug=False, num_devices=NCORES)

    d = {}
    R_INPUTS = {"A4", "W1T4", "WT", "PACKR"}
    for name, shape in INPUT_SHAPES.items():
        dt = F32R if name in R_INPUTS else F32
        d[name] = nc.dram_tensor(name, list(shape), dt, kind="ExternalInput").ap()
    for name in ("OSEG", "OREC"):
        d[name] = nc.dram_tensor(name, [NPX], F32, kind="ExternalOutput").ap()

    with tile.TileContext(nc) as tc, ExitStack() as ctx:
        pers = ctx.enter_context(tc.tile_pool(name="pers", bufs=1))
        pa = ctx.enter_context(tc.tile_pool(name="act", bufs=3))
        pt16 = ctx.enter_context(tc.tile_pool(name="t16", bufs=2))
        pt1 = ctx.enter_context(tc.tile_pool(name="t1", bufs=2))
        ppc = ctx.enter_context(tc.tile_pool(name="ppc", bufs=4, space="PSUM"))
        pps = ctx.enter_context(tc.tile_pool(name="pps", bufs=4, space="PSUM"))

        # ---- persistent loads ----
        # W1T + packed constants go first on the sync queue (conv1 and the
        # post-pipeline need them early); WT streams on scalar+gpsimd queues
        # immediately; A arrives in column chunks so conv1 starts early.
        W1T4 = pers.tile([128, 256], F32R, tag="W1T4")
        nc.sync.dma_start(W1T4[:], d["W1T4"][:])
        PACKR = pers.tile([128, 370], F32R, tag="PACKR")
        nc.sync.dma_start(PACKR[:], d["PACKR"][:])
        PACKF = pers.tile([128, 9], F32, tag="PACKF")
        nc.sync.dma_start(PACKF[:], d["PACKF"][:])

        WsT = PACKR[:, 0:16]
        INDSQ = PACKR[:, 16:32]
        IND2 = PACKR[0:16, 32:160]
        WR1T = PACKR[0:16, 160:224]
        WR2T = PACKR[0:64, 224:352]
        WR3T = PACKR[:, 352:353]
        ONES16 = PACKR[0:16, 353:354]
        ONES1x16 = PACKR[0:1, 354:370]
        CB1 = PACKF[:, 0:2]
        ZERO128 = PACKF[:, 2:3]
        BR1 = PACKF[0:64, 3:4]
        BR2 = PACKF[:, 4:5]
        BR3 = PACKF[0:1, 5:6]
        CB2 = PACKF[0:16, 6:7]
        EPS16 = PACKF[0:16, 7:8]
        EPS1 = PACKF[0:1, 8:9]

        A4 = pers.tile([128, AFLAT // 4], F32R, tag="A4")
        nc.sync.dma_start(A4[:], d["A4"][:])

        WT = pers.tile([128, 2, 25, 256], F32R, tag="WT")
        _dma_engines = [nc.scalar, nc.gpsimd, nc.sync]
        for k in range(2):
            for t in range(25):
                eng = _dma_engines[(k * 25 + t) % len(_dma_engines)]
                eng.dma_start(WT[:, k, t, :], d["WT"][k, t, :, :])

        C1 = [pers.tile([128, AFLAT], F32R, tag=f"C1_{m}", name=f"C1_{m}")
              for m in range(2)]

        # ---- conv1: 1->256 5x5 via host im2col (25 taps + valid-mask + bias
        # rows). A is stacked as 4 column-quarters on partition groups
        # {0,32,64,96} so the four matmuls row-tile the PE concurrently. ----
        QW = AFLAT // 4
        for m in range(2):
            for qoff in range(0, QW, 512):
                n = min(512, QW - qoff)
                for qt in range(4):
                    ps = ppc.tile([128, 512], F32, tag="ppc")
                    nc.tensor.matmul(
                        ps[:, :n],
                        W1T4[32 * qt:32 * qt + 27, m * 128:(m + 1) * 128],
                        A4[32 * qt:32 * qt + 27, qoff:qoff + n],
                        start=True, stop=True,
                        tile_position=(32 * qt, 0),
                    )
                    nc.scalar.activation(
                        C1[m][:, QW * qt + qoff:QW * qt + qoff + n], ps[:, :n],
                        AF.Relu, bias=ZERO128[:], scale=1.0,
                    )

        C13 = [C1[m][:].rearrange("p (r c) -> p r c", c=CW) for m in range(2)]

        # ---- main loop over 8 N-tiles of 512 px (4 output rows each) ----
        for t in range(NT):
            px = slice(512 * t, 512 * (t + 1))
            prim = []
            for m in range(2):
                # primary caps conv: 50-matmul accumulation chain
                ps = ppc.tile([128, 512], F32, tag="ppc")
                idx = 0
                for k in range(2):
                    for dy in range(5):
                        for dx in range(5):
                            nc.tensor.matmul(
                                ps[:],
                                WT[:, k, dy * 5 + dx, m * 128:(m + 1) * 128],
                                C13[k][:, 4 * t + dy:4 * t + dy + 4, dx:dx + 128],
                                start=(idx == 0), stop=(idx == 49),
                            )
                            idx += 1
                # preact = psum/32 + (bp/32 + cbp)
                P = pa.tile([128, 512], F32, tag="P")
                nc.scalar.activation(P[:], ps[:], AF.Identity,
                                     bias=CB1[:, m:m + 1], scale=1.0 / 32.0)
                # squash over each capsule's 8 atoms (partition groups)
                S = pa.tile([128, 512], F32R, tag="S")
                nc.vector.tensor_mul(out=S[:], in0=P[:], in1=P[:])
                sq = pps.tile([128, 512], F32, tag="pps")
                nc.tensor.matmul(sq[:16, :], INDSQ[:], S[:],
                                 start=True, stop=True)
                tq = pt16.tile([16, 512], F32, tag="tq")
                nc.scalar.activation(tq[:], sq[:16, :], AF.Sqrt, bias=EPS16[:], scale=1.0)
                u = pt16.tile([16, 512], F32, tag="u")
                nc.vector.tensor_mul(out=u[:], in0=sq[:16, :], in1=tq[:])
                nc.vector.tensor_add(out=u[:], in0=u[:], in1=tq[:])
                rf0 = pt16.tile([16, 512], F32, tag="rf0")
                nc.vector.reciprocal_approx_fast(out=rf0[:], in_=u[:])
                rf = pt16.tile([16, 512], F32R, tag="rf")
                nc.vector.tensor_mul(out=rf[:], in0=sq[:16, :], in1=rf0[:])
                bc = pps.tile([128, 512], F32, tag="pps")
                nc.tensor.matmul(bc[:], IND2[:], rf[:],
                                 start=True, stop=True)
                pm = pa.tile([128, 512], F32R, tag="prim")
                nc.vector.tensor_mul(out=pm[:], in0=P[:], in1=bc[:])
                prim.append(pm)

            # seg votes + sum over 32 input capsules, both chunks into one psum
            spp = pps.tile([128, 512], F32, tag="pps")
            nc.tensor.matmul(spp[:16, :], WsT[:], prim[0][:],
                             start=True, stop=False)
            nc.tensor.matmul(spp[:16, :], WsT[:], prim[1][:],
                             start=False, stop=True)
            sp = pt16.tile([16, 512], F32, tag="sp")
            nc.scalar.activation(sp[:], spp[:16, :], AF.Identity, bias=CB2[:], scale=1.0)

            # seg squash scalar factor from sq2 = sum_a sp^2
            sp2 = pt16.tile([16, 512], F32R, tag="sp2")
            nc.vector.tensor_mul(out=sp2[:], in0=sp[:], in1=sp[:])
            sq2p = pps.tile([128, 512], F32, tag="pps")
            nc.tensor.matmul(sq2p[:1, :], ONES16[:], sp2[:],
                             start=True, stop=True)
            t2 = pt1.tile([1, 512], F32, tag="t2")
            nc.scalar.activation(t2[:], sq2p[:1, :], AF.Sqrt, bias=EPS1[:], scale=1.0)
            u2 = pt1.tile([1, 512], F32, tag="u2")
            nc.vector.tensor_mul(out=u2[:], in0=sq2p[:1, :], in1=t2[:])
            nc.vector.tensor_add(out=u2[:], in0=u2[:], in1=t2[:])
            f2 = pt1.tile([1, 512], F32, tag="f2")
            nc.vector.reciprocal_approx_fast(out=f2[:], in_=u2[:])
            nc.vector.tensor_mul(out=f2[:], in0=sq2p[:1, :], in1=f2[:])

            # out_seg = sqrt(f2^2 * sq2 + eps)
            q = pt1.tile([1, 512], F32, tag="q")
            nc.vector.tensor_mul(out=q[:], in0=f2[:], in1=f2[:])
            nc.vector.tensor_mul(out=q[:], in0=q[:], in1=sq2p[:1, :])
            oseg = pt1.tile([1, 512], F32, tag="oseg")
            nc.scalar.activation(oseg[:], q[:], AF.Sqrt, bias=EPS1[:], scale=1.0)
            nc.sync.dma_start(d["OSEG"][px].rearrange("(p n) -> p n", p=1), oseg[:])

            # masked = sp * (f2 * y), broadcast over the 16 atoms
            yt = pt1.tile([1, 512], F32, tag="yt")
            nc.sync.dma_start(yt[:], d["YV"][px].rearrange("(p n) -> p n", p=1))
            m1 = pt1.tile([1, 512], F32R, tag="m1")
            nc.vector.tensor_mul(out=m1[:], in0=f2[:], in1=yt[:])
            bmp = pps.tile([128, 512], F32, tag="pps")
            nc.tensor.matmul(bmp[:16, :], ONES1x16[:], m1[:],
                             start=True, stop=True)
            masked = pt16.tile([16, 512], F32R, tag="masked")
            nc.vector.tensor_mul(out=masked[:], in0=sp[:], in1=bmp[:16, :])

            # recon: 16 -> 64 -> 128 -> 1 (1x1 convs)
            r1p = pps.tile([128, 512], F32, tag="pps")
            nc.tensor.matmul(r1p[:64, :], WR1T[:], masked[:],
                             start=True, stop=True)
            r1 = pa.tile([64, 512], F32R, tag="r1")
            nc.scalar.activation(r1[:], r1p[:64, :], AF.Relu, bias=BR1[:], scale=1.0)
            r2p = pps.tile([128, 512], F32, tag="pps")
            nc.tensor.matmul(r2p[:], WR2T[:], r1[:],
                             start=True, stop=True)
            r2 = pa.tile([128, 512], F32R, tag="r2")
            nc.scalar.activation(r2[:], r2p[:], AF.Relu, bias=BR2[:], scale=1.0)
            r3p = pps.tile([128, 512], F32, tag="pps")
            nc.tensor.matmul(r3p[:1, :], WR3T[:], r2[:],
                             start=True, stop=True)
            orec = pt1.tile([1, 512], F32, tag="orec")
            nc.scalar.activation(orec[:], r3p[:1, :], AF.Sigmoid, bias=BR3[:], scale=1.0)
            nc.sync.dma_start(d["OREC"][px].rearrange("(p n) -> p n", p=1), orec[:])

    nc.compile()
    return nc


def _get_program():
    global _PROGRAM
    if _PROGRAM is None:
        _PROGRAM = _build_program()
    return _PROGRAM


def _host_prep(inputs):
    """Build per-core input maps from the full problem inputs."""
    x = np.asarray(inputs["x"], np.float32)
    y = np.asarray(inputs["y"], np.float32)
    W1 = np.asarray(inputs["W1"], np.float32)
    b1 = np.asarray(inputs["b1"], np.float32)
    Wp = np.asarray(inputs["Wp"], np.float32)
    bp = np.asarray(inputs["bp"], np.float32)
    cbp = np.asarray(inputs["cbp"], np.float32)
    Ws = np.asarray(inputs["Ws"], np.float32)
    bs = np.asarray(inputs["bs"], np.float32)
    cbs = np.asarray(inputs["cbs"], np.float32)
    Wr1 = np.asarray(inputs["Wr1"], np.float32)
    br1 = np.asarray(inputs["br1"], np.float32)
    Wr2 = np.asarray(inputs["Wr2"], np.float32)
    br2 = np.asarray(inputs["br2"], np.float32)
    Wr3 = np.asarray(inputs["Wr3"], np.float32)
    br3 = np.asarray(inputs["br3"], np.float32)

    W1r = W1.reshape(256, 25).T                      # [25 tap, 256 oc]
    W1T = np.concatenate([W1r, np.ones((1, 256), np.float32),
                          b1[None, :]], axis=0)      # [27, 256]
    W1T4 = np.zeros((128, 256), np.float32)
    for qt in range(4):
        W1T4[32 * qt:32 * qt + 27] = W1T
    WT = np.ascontiguousarray(
        Wp.reshape(256, 2, 128, 25).transpose(1, 3, 2, 0))  # [2k, 25tap, 128p, 256oc]

    oc = np.arange(128)
    WsT = np.ascontiguousarray(Ws.reshape(16, 8).T[oc % 8])       # [128, 16]
    IND2 = (np.arange(128)[None, :] // 8 == np.arange(16)[:, None]).astype(np.float32)
    INDSQ = np.ascontiguousarray(IND2.T)
    cb1 = np.empty((128, 2), np.float32)
    for m in range(2):
        g = m * 128 + np.arange(128)
        cb1[:, m] = bp[g] / 32.0 + cbp[g // 8, g % 8, 0, 0]
    cb2 = (32.0 * bs + cbs[0, :, 0, 0]).astype(np.float32)[:, None]

    packr = np.zeros((128, 370), np.float32)
    packr[:, 0:16] = WsT
    packr[:, 16:32] = INDSQ
    packr[0:16, 32:160] = IND2
    packr[0:16, 160:224] = Wr1.reshape(64, 16).T
    packr[0:64, 224:352] = Wr2.reshape(128, 64).T
    packr[:, 352:353] = Wr3.reshape(1, 128).T
    packr[0:16, 353:354] = 1.0
    packr[0:1, 354:370] = 1.0
    packf = np.zeros((128, 9), np.float32)
    packf[:, 0:2] = cb1
    packf[0:64, 3] = br1
    packf[:, 4] = br2
    packf[0, 5] = br3[0]
    packf[0:16, 6] = cb2[:, 0]
    packf[0:16, 7] = 1e-9
    packf[0, 8] = 1e-9
    shared = {
        "W1T4": W1T4,
        "WT": WT,
        "PACKR": packr,
        "PACKF": packf,
    }

    in_maps = []
    for c in range(NCORES):
        b, j = divmod(c, NBLK)
        r0 = RB * j
        xpad = np.zeros((H + 8, W + 8), np.float32)
        xpad[4:4 + H, 4:4 + W] = x[b, 0]
        A = np.empty((27, RR, CW), np.float32)
        for dy in range(5):
            for dx in range(5):
                A[dy * 5 + dx] = xpad[r0 + dy:r0 + dy + RR, dx:dx + CW]
        # valid-mask row: -1e30 where the conv1 output position is padding
        rr = np.arange(RR)[:, None]
        cc = np.arange(CW)[None, :]
        valid = (r0 - 2 + rr >= 0) & (r0 - 2 + rr < H) & (cc >= 2) & (cc < 2 + W)
        A[25] = np.where(valid, 0.0, -1e30).astype(np.float32)
        A[26] = 1.0
        m = dict(shared)
        Af = A.reshape(27, AFLAT)
        A4 = np.zeros((128, AFLAT // 4), np.float32)
        for qt in range(4):
            A4[32 * qt:32 * qt + 27] = Af[:, (AFLAT // 4) * qt:(AFLAT // 4) * (qt + 1)]
        m["A4"] = A4
        m["YV"] = np.ascontiguousarray(y[b, 0, r0:r0 + RB, :].reshape(NPX))
        in_maps.append(m)
    return in_maps


def _gather(results):
    out_seg = np.empty((B, 1, H, W), np.float32)
    out_rec = np.empty((B, 1, H, W), np.float32)
    for c in range(NCORES):
        b, j = divmod(c, NBLK)
        r0 = RB * j
        out_seg[b, 0, r0:r0 + RB, :] = results[c]["OSEG"].reshape(RB, W)
        out_rec[b, 0, r0:r0 + RB, :] = results[c]["OREC"].reshape(RB, W)
    return out_seg, out_rec


def kernel(**inputs):
    nc = _get_program()
    in_maps = _host_prep(inputs)
    res = run_bass_kernel_spmd(nc, in_maps, list(range(NCORES)))
    return _gather(res.results)
